# revision 7
# baseline (speedup 1.0000x reference)
"""Criss-cross attention (CCNet) kernel for 8 TRN2 NeuronCores.

Data-parallel over batch N=8: one image per core. Per image (512ch, 96x96):
  t/f = 1x1 conv to 64ch -> row/col affinities -> exp -> denominators ->
  reciprocal folded into Wr/Wc -> aggregate g' = (inc_w@g_w)@x along rows
  and cols -> two spatial-major outputs, recombined with the residual on
  host.

Key folds vs the reference:
  - inc_w @ g_w is precomputed on host (M), so the g conv and inc conv
    become ONE conv of the aggregated input (softmax weights sum to 1, so
    g_b folds into a host-side bias).
  - All convs run in fp8e4 DoubleRow (2 contraction chunks per matmul).
    x ships only as fp8. M is prescaled by 512 (outputs divided on host);
    t/f weights+biases are prescaled by 256 and the exp's free input
    scale divides the affinities by 256^2.
  - softmax normalization (1/denominator) is multiplied into the attention
    weights before aggregation, so aggregation outputs are final.
  - Aggregation keeps the attention line stationary and streams g' (one
    N=512 matmul per line), producing spatial-major outputs.
  - residual + biases are added on host; outputs ship as bf16.
"""

import sys

sys.path.insert(0, "/opt/trn_rl_repo")

from contextlib import ExitStack

import numpy as np
import ml_dtypes

import concourse.bass as bass
import concourse.bacc as bacc
import concourse.tile as tile
from concourse import mybir
from concourse.bass_utils import run_bass_kernel_spmd

BF16 = mybir.dt.bfloat16
F32 = mybir.dt.float32
FP8 = mybir.dt.float8e4
AF = mybir.ActivationFunctionType
DR = mybir.MatmulPerfMode.DoubleRow

N, C_IN, C_INNER, C_OUT, H, W = 8, 512, 64, 512, 96, 96
HW = H * W  # 9216
KC = C_IN // 128  # 4 contraction chunks
MSCALE = 512.0   # fp8 prescale of M = inc_w @ g_w
TSCALE = 256.0   # fp8 prescale of t/f weights+biases

_cache = {}


def build_program():
    nc = bacc.Bacc()

    # ---- DRAM I/O ----
    xf8_d = nc.dram_tensor("x_f8", (128, KC, HW), FP8, kind="ExternalInput")
    tfw_d = nc.dram_tensor("tf_wT", (128, KC, 128), FP8, kind="ExternalInput")
    tfw2_d = nc.dram_tensor("tf_wT2", (128, KC, 128), FP8, kind="ExternalInput")
    mw_d = nc.dram_tensor("m_wT", (128, KC, C_OUT), FP8, kind="ExternalInput")
    tfb_d = nc.dram_tensor("tf_b", (128, 1), F32, kind="ExternalInput")
    tfb2_d = nc.dram_tensor("tf_b2", (128, 1), F32, kind="ExternalInput")
    mask_d = nc.dram_tensor("mask", (96, 96), BF16, kind="ExternalInput")
    ones96b_d = nc.dram_tensor("ones96b", (96, 128), BF16, kind="ExternalInput")
    # spatial-major outputs: [line, pos-in-line, channel]
    ocol_d = nc.dram_tensor("out_col", (W, H, C_OUT), BF16, kind="ExternalOutput")
    orow_d = nc.dram_tensor("out_row", (H, W, C_OUT), BF16, kind="ExternalOutput")

    with ExitStack() as ctx:
        tc = ctx.enter_context(tile.TileContext(nc))
        p0 = ctx.enter_context(tc.tile_pool(name="p0", bufs=1))

        # ---- persistent tiles ----
        Xf8 = p0.tile([128, KC, H, W], FP8)  # channel-major image, fp8
        ones96b = p0.tile([96, 128], BF16)
        mask = p0.tile([96, 96], BF16)
        mw = p0.tile([128, KC, C_OUT], FP8)
        TFb = p0.tile([128, 2, H, W], BF16)  # t/f activations (doubled, x256)
        Wr = p0.tile([96, H, W], BF16)       # exp row affin: Wr[i, y, x]
        Wc = p0.tile([96, W, H], BF16)       # exp col affin: Wc[j, x, y]
        rr = p0.tile([128, W, H], BF16)      # 1/denominator, [*, x, y] bcast

        nc.sync.dma_start(out=ones96b, in_=ones96b_d[:])
        nc.sync.dma_start(out=mask, in_=mask_d[:])
        nc.sync.dma_start(out=mw, in_=mw_d[:])
        xv8 = xf8_d[:].rearrange("p a (h w) -> p a h w", h=H)
        for q in range(8):
            for k in range(KC):
                nc.sync.dma_start(
                    out=Xf8[:, k, q * 12:(q + 1) * 12, :],
                    in_=xv8[:, k, q * 12:(q + 1) * 12, :])

        T = TFb[0:64, 0]
        F = TFb[0:64, 1]
        T2 = TFb[64:128, 0]
        F2 = TFb[64:128, 1]

        # ---- phase 1: t/f conv (fp8 DoubleRow, values x TSCALE) ----
        with tc.tile_pool(name="pe", bufs=1) as pe, \
             tc.tile_pool(name="ptf_ps", bufs=2, space="PSUM") as ptf_ps:
            tfw = pe.tile([128, KC, 128], FP8)
            tfw2 = pe.tile([128, KC, 128], FP8)
            tb = pe.tile([64, 1], F32)
            fb = pe.tile([64, 1], F32)
            tfbF = pe.tile([128, 1], F32)
            tb2 = pe.tile([128, 1], F32)
            nc.sync.dma_start(out=tfw, in_=tfw_d[:])
            nc.sync.dma_start(out=tfw2, in_=tfw2_d[:])
            nc.sync.dma_start(out=tb, in_=tfb_d[0:64])
            nc.sync.dma_start(out=fb, in_=tfb_d[64:128])
            nc.sync.dma_start(out=tfbF, in_=tfb_d[:])
            nc.sync.dma_start(out=tb2, in_=tfb2_d[:])

            Xflat = Xf8.rearrange("p a h w -> p a (h w)")
            Tflat = TFb.rearrange("p c h w -> p c (h w)")
            for b in range(HW // 512):
                sl = slice(b * 512, (b + 1) * 512)
                pst = ptf_ps.tile([128, 512], F32, tag="pt")
                psf = ptf_ps.tile([128, 512], F32, tag="pf")
                for k in range(2):
                    nc.tensor.matmul(
                        pst, tfw[:, 2 * k:2 * k + 2, :],
                        Xflat[:, 2 * k:2 * k + 2, sl],
                        start=(k == 0), stop=(k == 1), perf_mode=DR)
                for k in range(2):
                    nc.tensor.matmul(
                        psf, tfw2[:, 2 * k:2 * k + 2, :],
                        Xflat[:, 2 * k:2 * k + 2, sl],
                        start=(k == 0), stop=(k == 1), perf_mode=DR)
                # pst = [t; f] stacked, psf = [f; t] stacked
                nc.scalar.activation(Tflat[0:64, 0, sl], pst[0:64],
                                     AF.Identity, bias=tb)
                nc.vector.tensor_scalar_add(Tflat[64:128, 1, sl], pst[64:128],
                                            tfbF[64:128])
                nc.scalar.activation(Tflat[0:64, 1, sl], psf[0:64],
                                     AF.Identity, bias=fb)
                nc.vector.tensor_scalar_add(Tflat[64:128, 0, sl], psf[64:128],
                                            tb2[64:128])

        # ---- phase 2: affinities + exp (+ mask on cols) ----
        # Wr[i, y, x] = exp(sum_c f[c,y,i] t[c,y,x] / TSCALE^2); 8-row blocks.
        ESC = 1.0 / (TSCALE * TSCALE)
        Wr8 = Wr.rearrange("i (b four two) w -> i b four two w", four=4, two=2)
        Wc8 = Wc.rearrange("j (b four two) y -> j b four two y", four=4, two=2)
        with tc.tile_pool(name="pe_ps", bufs=2, space="PSUM") as pe_ps:
            for y0 in range(0, H, 8):
                psA = pe_ps.tile([96, 4, 96], F32, tag="peA")
                psB = pe_ps.tile([96, 4, 96], F32, tag="peB")
                for h in range(4):
                    nc.tensor.matmul(psA[:, h, :], F[:, y0 + 2 * h, :],
                                     T[:, y0 + 2 * h, :], start=True, stop=True)
                    nc.tensor.matmul(psB[:, h, :], F2[:, y0 + 2 * h + 1, :],
                                     T2[:, y0 + 2 * h + 1, :],
                                     start=True, stop=True)
                b = y0 // 8
                nc.scalar.activation(Wr8[:, b, :, 0, :], psA, AF.Exp, scale=ESC)
                nc.scalar.activation(Wr8[:, b, :, 1, :], psB, AF.Exp, scale=ESC)
            # col affinities: Wc[j, x, y] = exp(.../TSCALE^2), j != y
            mb8 = bass.AP(tensor=mask.tensor, offset=mask.offset,
                          ap=[mask.ap[0], [0, 8], mask.ap[1]])
            for x0 in range(0, W, 8):
                psA = pe_ps.tile([96, 4, 96], F32, tag="peA")
                psB = pe_ps.tile([96, 4, 96], F32, tag="peB")
                for h in range(4):
                    nc.tensor.matmul(psA[:, h, :], F[:, :, x0 + 2 * h],
                                     T[:, :, x0 + 2 * h], start=True, stop=True)
                    nc.tensor.matmul(psB[:, h, :], F2[:, :, x0 + 2 * h + 1],
                                     T2[:, :, x0 + 2 * h + 1],
                                     start=True, stop=True)
                b = x0 // 8
                nc.scalar.activation(Wc8[:, b, :, 0, :], psA, AF.Exp, scale=ESC)
                nc.scalar.activation(Wc8[:, b, :, 1, :], psB, AF.Exp, scale=ESC)
                wcs = Wc[:, x0:x0 + 8, :]
                nc.gpsimd.tensor_mul(wcs, wcs, mb8)

        # ---- phase 3: denominators ([x, y] layout) -> 1/D -> fold into W ----
        WrT = Wr.rearrange("i h w -> i w h")
        with tc.tile_pool(name="pd_ps", bufs=4, space="PSUM") as pd_ps, \
             tc.tile_pool(name="prf", bufs=3) as prf:
            for x0 in range(0, W, 4):
                xs = slice(x0, x0 + 4)
                psd = pd_ps.tile([128, 4, 96], F32, tag="pd")
                nc.tensor.matmul(psd, ones96b, Wc[:, xs, :],
                                 start=True, stop=False)
                nc.tensor.matmul(psd, ones96b, WrT[:, xs, :],
                                 start=False, stop=True)
                rf = prf.tile([128, 4, 96], F32, tag="rf")
                nc.vector.reciprocal_approx_fast(out=rf, in_=psd)
                nc.vector.tensor_copy(rr[:, xs, :], rf)
            # normalize: Wc[j, x, y] *= rr[x, y]; Wr[i, y, x] *= rr[x, y]
            rrv = rr[0:96]
            rrT = rrv.rearrange("p x y -> p y x")
            for x0 in range(0, W, 8):
                xs = slice(x0, x0 + 8)
                nc.gpsimd.tensor_mul(Wc[:, xs, :], Wc[:, xs, :], rrv[:, xs, :])
            for y0 in range(0, H, 8):
                ys = slice(y0, y0 + 8)
                nc.gpsimd.tensor_mul(Wr[:, ys, :], Wr[:, ys, :], rrT[:, ys, :])

        # ---- phases 4/5: conv g'=(M@x) fp8-DoubleRow + aggregate ----
        with tc.tile_pool(name="pg", bufs=4) as pg, \
             tc.tile_pool(name="pu", bufs=3) as pu, \
             tc.tile_pool(name="pg_ps", bufs=2, space="PSUM") as pg_ps, \
             tc.tile_pool(name="pu_ps", bufs=2, space="PSUM") as pu_ps:
            for mode in ("col", "row"):
                Wa = Wc if mode == "col" else Wr
                out_d = ocol_d if mode == "col" else orow_d
                for s0 in range(0, 96, 2):
                    # g' conv for 2 lines -> psg2[:, i, :]
                    psg2 = pg_ps.tile([96, 2, C_OUT], F32, tag="pg")
                    for i in range(2):
                        for k in range(2):
                            lhs = (Xf8[:, 2 * k:2 * k + 2, :, s0 + i]
                                   if mode == "col"
                                   else Xf8[:, 2 * k:2 * k + 2, s0 + i, :])
                            nc.tensor.matmul(psg2[:, i, :], lhs,
                                             mw[:, 2 * k:2 * k + 2, :],
                                             start=(k == 0), stop=(k == 1),
                                             perf_mode=DR)
                    gt = pg.tile([96, 2, C_OUT], BF16, tag="gt")
                    if (s0 // 2) % 2 == 0:
                        nc.scalar.copy(gt, psg2)
                    else:
                        nc.vector.tensor_copy(gt, psg2)
                    # aggregation: psu2[:, i, :] = Wa[:, s0+i, :].T @ gt[:, i, :]
                    psu2 = pu_ps.tile([96, 2, C_OUT], F32, tag="pu")
                    for i in range(2):
                        nc.tensor.matmul(psu2[:, i, :], Wa[:, s0 + i, :],
                                         gt[:, i, :], start=True, stop=True)
                    uc = pu.tile([96, 2, C_OUT], BF16, tag="uc")
                    if (s0 // 2) % 2 == 1:
                        nc.scalar.copy(uc, psu2)
                    else:
                        nc.vector.tensor_copy(uc, psu2)
                    nc.sync.dma_start(
                        out=out_d[s0:s0 + 2].rearrange("l p c -> p l c"),
                        in_=uc)

    nc.finalize()
    return nc


def _prep_shared(t_w, t_b, f_w, f_b, g_w, g_b, inc_w, inc_b):
    bf = ml_dtypes.bfloat16
    f8 = ml_dtypes.float8_e4m3
    m_w = inc_w @ g_w  # (C_IN, C_IN) fold: inc(g(.)) == M @ .
    s = TSCALE
    d = {
        "tf_wT": np.ascontiguousarray(
            np.concatenate([t_w.T * s, f_w.T * s], axis=1)
            .reshape(KC, 128, 128).transpose(1, 0, 2)).astype(f8),
        "tf_wT2": np.ascontiguousarray(
            np.concatenate([f_w.T * s, t_w.T * s], axis=1)
            .reshape(KC, 128, 128).transpose(1, 0, 2)).astype(f8),
        "m_wT": np.ascontiguousarray(
            (m_w.T * MSCALE).reshape(KC, 128, C_OUT)
            .transpose(1, 0, 2)).astype(f8),
        "tf_b": (np.concatenate([t_b, f_b]) * s).reshape(128, 1)
            .astype(np.float32),
        "tf_b2": (np.concatenate([f_b, t_b]) * s).reshape(128, 1)
            .astype(np.float32),
        "mask": (1.0 - np.eye(96)).astype(bf),
        "ones96b": np.ones((96, 128), bf),
    }
    comb_b = (inc_b + inc_w @ g_b).astype(np.float32)  # host-side bias
    return d, comb_b


def _make_in_maps(inputs):
    x = np.asarray(inputs["x"], dtype=np.float32)
    shared, comb_b = _prep_shared(*[
        np.asarray(inputs[k], np.float32)
        for k in ("t_w", "t_b", "f_w", "f_b", "g_w", "g_b", "inc_w", "inc_b")])
    _cache["comb_b"] = comb_b
    _cache["x"] = x
    f8 = ml_dtypes.float8_e4m3
    in_maps = []
    for n in range(N):
        xi = x[n].reshape(KC, 128, HW)  # (4, 128, 9216)
        m = dict(shared)
        m["x_f8"] = np.ascontiguousarray(xi.transpose(1, 0, 2)).astype(f8)
        in_maps.append(m)
    return in_maps


def _post(results):
    x = _cache["x"]
    comb_b = _cache["comb_b"]
    inv = 1.0 / MSCALE
    out = np.empty((N, C_IN, H, W), np.float32)
    for n in range(N):
        row = results[n]["out_row"].astype(np.float32)  # (H, W, C)
        col = results[n]["out_col"].astype(np.float32)  # (W, H, C)
        agg = row.transpose(2, 0, 1) + col.transpose(2, 1, 0)
        out[n] = x[n] + comb_b[:, None, None] + agg * inv
    return out


def kernel(x, t_w, t_b, f_w, f_b, g_w, g_b, inc_w, inc_b):
    in_maps = _make_in_maps(dict(
        x=x, t_w=t_w, t_b=t_b, f_w=f_w, f_b=f_b, g_w=g_w, g_b=g_b,
        inc_w=inc_w, inc_b=inc_b))

    if "nc" not in _cache:
        _cache["nc"] = build_program()
    res = run_bass_kernel_spmd(_cache["nc"], in_maps, core_ids=list(range(N)))
    return _post(res.results)


if __name__ == "__main__":
    rng = np.random.default_rng(0)
    ins = {
        "x": rng.standard_normal((N, C_IN, H, W), dtype=np.float32),
        "t_w": rng.standard_normal((C_INNER, C_IN), dtype=np.float32) * 0.02,
        "t_b": np.zeros(C_INNER, np.float32),
        "f_w": rng.standard_normal((C_INNER, C_IN), dtype=np.float32) * 0.02,
        "f_b": np.zeros(C_INNER, np.float32),
        "g_w": rng.standard_normal((C_OUT, C_IN), dtype=np.float32) * 0.02,
        "g_b": np.zeros(C_OUT, np.float32),
        "inc_w": rng.standard_normal((C_IN, C_OUT), dtype=np.float32) * 0.02,
        "inc_b": np.zeros(C_IN, np.float32),
    }
    y = kernel(**ins)
    print(y.shape, y.dtype)


# revision 12
# speedup vs baseline: 1.1077x; 1.1077x over previous
"""Criss-cross attention (CCNet) kernel for 8 TRN2 NeuronCores.

Data-parallel over batch N=8: one image per core. Per image (512ch, 96x96):
  t/f = 1x1 conv to 64ch -> row/col affinities -> exp -> denominators ->
  reciprocal folded into Wr/Wc -> aggregate g' = (inc_w@g_w)@x along rows
  and cols -> two spatial-major outputs, recombined with the residual on
  host.

Key folds vs the reference:
  - inc_w @ g_w is precomputed on host (M), so the g conv and inc conv
    become ONE conv of the aggregated input (softmax weights sum to 1, so
    g_b folds into a host-side bias).
  - All convs run in fp8e4 DoubleRow (2 contraction chunks per matmul).
    x ships only as fp8. M is prescaled by 512 (outputs divided on host);
    t/f weights+biases are prescaled by 256 and the exp's free input
    scale divides the affinities by 256^2.
  - softmax normalization (1/denominator) is multiplied into the attention
    weights before aggregation, so aggregation outputs are final.
  - Aggregation keeps the attention line stationary and streams g' (one
    N=512 matmul per line), producing spatial-major outputs.
  - residual + biases are added on host; outputs ship as bf16.
"""

import sys

sys.path.insert(0, "/opt/trn_rl_repo")

from contextlib import ExitStack

import numpy as np
import ml_dtypes

import concourse.bass as bass
import concourse.bacc as bacc
import concourse.tile as tile
from concourse import mybir
from concourse.bass_utils import run_bass_kernel_spmd

BF16 = mybir.dt.bfloat16
F32 = mybir.dt.float32
FP8 = mybir.dt.float8e4
AF = mybir.ActivationFunctionType
DR = mybir.MatmulPerfMode.DoubleRow

N, C_IN, C_INNER, C_OUT, H, W = 8, 512, 64, 512, 96, 96
HW = H * W  # 9216
KC = C_IN // 128  # 4 contraction chunks
MSCALE = 512.0   # fp8 prescale of M = inc_w @ g_w
TSCALE = 256.0   # fp8 prescale of t/f weights+biases

_cache = {}


def build_program():
    nc = bacc.Bacc()

    # ---- DRAM I/O ----
    xf8_d = nc.dram_tensor("x_f8", (128, KC, HW), FP8, kind="ExternalInput")
    tfw_d = nc.dram_tensor("tf_wT", (128, KC, 128), FP8, kind="ExternalInput")
    tfw2_d = nc.dram_tensor("tf_wT2", (128, KC, 128), FP8, kind="ExternalInput")
    mw_d = nc.dram_tensor("m_wT", (128, KC, C_OUT), FP8, kind="ExternalInput")
    tfb_d = nc.dram_tensor("tf_b", (128, 1), F32, kind="ExternalInput")
    tfb2_d = nc.dram_tensor("tf_b2", (128, 1), F32, kind="ExternalInput")
    mask_d = nc.dram_tensor("mask", (96, 96), BF16, kind="ExternalInput")
    ones96b_d = nc.dram_tensor("ones96b", (96, 128), BF16, kind="ExternalInput")
    # spatial-major outputs: [line, pos-in-line, channel]
    ocol_d = nc.dram_tensor("out_col", (W, H, C_OUT), BF16, kind="ExternalOutput")
    orow_d = nc.dram_tensor("out_row", (H, W, C_OUT), BF16, kind="ExternalOutput")

    with ExitStack() as ctx:
        tc = ctx.enter_context(tile.TileContext(nc))
        p0 = ctx.enter_context(tc.tile_pool(name="p0", bufs=1))

        # ---- persistent tiles ----
        Xf8 = p0.tile([128, KC, H, W], FP8)  # channel-major image, fp8
        ones96b = p0.tile([96, 128], BF16)
        mask = p0.tile([96, 96], BF16)
        mw = p0.tile([128, KC, C_OUT], FP8)
        TFb = p0.tile([128, 2, H, W], BF16)  # t/f activations (doubled, x256)
        Wr = p0.tile([96, H, W], BF16)       # exp row affin: Wr[i, y, x]
        Wc = p0.tile([96, W, H], BF16)       # exp col affin: Wc[j, x, y]
        rr = p0.tile([128, W, H], BF16)      # 1/denominator, [*, x, y] bcast

        nc.sync.dma_start(out=ones96b, in_=ones96b_d[:])
        nc.sync.dma_start(out=mask, in_=mask_d[:])
        nc.sync.dma_start(out=mw, in_=mw_d[:])
        xv8 = xf8_d[:].rearrange("p a (h w) -> p a h w", h=H)

        T = TFb[0:64, 0]
        F = TFb[0:64, 1]
        T2 = TFb[64:128, 0]
        F2 = TFb[64:128, 1]

        # ---- phase 1: t/f conv (fp8 DoubleRow, values x TSCALE) ----
        with tc.tile_pool(name="pe", bufs=1) as pe, \
             tc.tile_pool(name="ptf_ps", bufs=2, space="PSUM") as ptf_ps:
            tfw = pe.tile([128, KC, 128], FP8)
            tfw2 = pe.tile([128, KC, 128], FP8)
            tb = pe.tile([64, 1], F32)
            fb = pe.tile([64, 1], F32)
            tfbF = pe.tile([128, 1], F32)
            tb2 = pe.tile([128, 1], F32)
            nc.sync.dma_start(out=tfw, in_=tfw_d[:])
            nc.sync.dma_start(out=tfw2, in_=tfw2_d[:])
            nc.sync.dma_start(out=tb, in_=tfb_d[0:64])
            nc.sync.dma_start(out=fb, in_=tfb_d[64:128])
            nc.sync.dma_start(out=tfbF, in_=tfb_d[:])
            nc.sync.dma_start(out=tb2, in_=tfb2_d[:])
            for q in range(8):
                for k in range(KC):
                    nc.sync.dma_start(
                        out=Xf8[:, k, q * 12:(q + 1) * 12, :],
                        in_=xv8[:, k, q * 12:(q + 1) * 12, :])

            Xflat = Xf8.rearrange("p a h w -> p a (h w)")
            Tflat = TFb.rearrange("p c h w -> p c (h w)")
            for b in range(HW // 512):
                sl = slice(b * 512, (b + 1) * 512)
                pst = ptf_ps.tile([128, 512], F32, tag="pt")
                psf = ptf_ps.tile([128, 512], F32, tag="pf")
                for k in range(2):
                    nc.tensor.matmul(
                        pst, tfw[:, 2 * k:2 * k + 2, :],
                        Xflat[:, 2 * k:2 * k + 2, sl],
                        start=(k == 0), stop=(k == 1), perf_mode=DR)
                for k in range(2):
                    nc.tensor.matmul(
                        psf, tfw2[:, 2 * k:2 * k + 2, :],
                        Xflat[:, 2 * k:2 * k + 2, sl],
                        start=(k == 0), stop=(k == 1), perf_mode=DR)
                # pst = [t; f] stacked, psf = [f; t] stacked
                nc.scalar.activation(Tflat[0:64, 0, sl], pst[0:64],
                                     AF.Identity, bias=tb)
                nc.vector.tensor_scalar_add(Tflat[64:128, 1, sl], pst[64:128],
                                            tfbF[64:128])
                nc.scalar.activation(Tflat[0:64, 1, sl], psf[0:64],
                                     AF.Identity, bias=fb)
                nc.vector.tensor_scalar_add(Tflat[64:128, 0, sl], psf[64:128],
                                            tb2[64:128])

        # ---- phase 2: affinities + exp (+ mask on cols) ----
        # Wr[i, y, x] = exp(sum_c f[c,y,i] t[c,y,x] / TSCALE^2); 8-row blocks.
        ESC = 1.0 / (TSCALE * TSCALE)
        Wr8 = Wr.rearrange("i (b four two) w -> i b four two w", four=4, two=2)
        Wc8 = Wc.rearrange("j (b four two) y -> j b four two y", four=4, two=2)
        with tc.tile_pool(name="pe_ps", bufs=2, space="PSUM") as pe_ps:
            for y0 in range(0, H, 8):
                psA = pe_ps.tile([96, 4, 96], F32, tag="peA")
                psB = pe_ps.tile([96, 4, 96], F32, tag="peB")
                for h in range(4):
                    nc.tensor.matmul(psA[:, h, :], F[:, y0 + 2 * h, :],
                                     T[:, y0 + 2 * h, :], start=True, stop=True)
                    nc.tensor.matmul(psB[:, h, :], F2[:, y0 + 2 * h + 1, :],
                                     T2[:, y0 + 2 * h + 1, :],
                                     start=True, stop=True)
                b = y0 // 8
                nc.scalar.activation(Wr8[:, b, :, 0, :], psA, AF.Exp, scale=ESC)
                nc.scalar.activation(Wr8[:, b, :, 1, :], psB, AF.Exp, scale=ESC)
            # col affinities: Wc[j, x, y] = exp(.../TSCALE^2), j != y
            mb8 = bass.AP(tensor=mask.tensor, offset=mask.offset,
                          ap=[mask.ap[0], [0, 8], mask.ap[1]])
            for x0 in range(0, W, 8):
                psA = pe_ps.tile([96, 4, 96], F32, tag="peA")
                psB = pe_ps.tile([96, 4, 96], F32, tag="peB")
                for h in range(4):
                    nc.tensor.matmul(psA[:, h, :], F[:, :, x0 + 2 * h],
                                     T[:, :, x0 + 2 * h], start=True, stop=True)
                    nc.tensor.matmul(psB[:, h, :], F2[:, :, x0 + 2 * h + 1],
                                     T2[:, :, x0 + 2 * h + 1],
                                     start=True, stop=True)
                b = x0 // 8
                nc.scalar.activation(Wc8[:, b, :, 0, :], psA, AF.Exp, scale=ESC)
                nc.scalar.activation(Wc8[:, b, :, 1, :], psB, AF.Exp, scale=ESC)
                wcs = Wc[:, x0:x0 + 8, :]
                nc.gpsimd.tensor_mul(wcs, wcs, mb8)

        # ---- phases 3-5: denominators -> 1/D -> fold into W, overlapped
        # with the g' conv + aggregation pipeline (agg lags conv by LAG
        # tiles so the PSUM->SBUF copy never blocks the tensor queue) ----
        WrT = Wr.rearrange("i h w -> i w h")
        rrv = rr[0:96]
        rrT = rrv.rearrange("p x y -> p y x")

        tiles = [("col", s0) for s0 in range(0, 96, 2)] + \
                [("row", s0) for s0 in range(0, 96, 2)]

        with tc.tile_pool(name="pg", bufs=26) as pg, \
             tc.tile_pool(name="pu", bufs=3) as pu, \
             tc.tile_pool(name="pg_ps", bufs=2, space="PSUM") as pg_ps:

            def emit_conv(idx):
                mode, s0 = tiles[idx]
                psg2 = pg_ps.tile([96, 2, C_OUT], F32, tag="pg")
                for i in range(2):
                    for k in range(2):
                        lhs = (Xf8[:, 2 * k:2 * k + 2, :, s0 + i]
                               if mode == "col"
                               else Xf8[:, 2 * k:2 * k + 2, s0 + i, :])
                        nc.tensor.matmul(psg2[:, i, :], lhs,
                                         mw[:, 2 * k:2 * k + 2, :],
                                         start=(k == 0), stop=(k == 1),
                                         perf_mode=DR)
                gt = pg.tile([96, 2, C_OUT], BF16, tag="gt")
                if idx % 2 == 0:
                    nc.scalar.copy(gt, psg2)
                else:
                    nc.vector.tensor_copy(gt, psg2)
                return gt

            def emit_agg(idx, gt, pu_ps):
                mode, s0 = tiles[idx]
                Wa = Wc if mode == "col" else Wr
                out_d = ocol_d if mode == "col" else orow_d
                psu2 = pu_ps.tile([96, 2, C_OUT], F32, tag="pu")
                for i in range(2):
                    nc.tensor.matmul(psu2[:, i, :], Wa[:, s0 + i, :],
                                     gt[:, i, :], start=True, stop=True)
                uc = pu.tile([96, 2, C_OUT], BF16, tag="uc")
                if idx % 2 == 1:
                    nc.scalar.copy(uc, psu2)
                else:
                    nc.vector.tensor_copy(uc, psu2)
                nc.sync.dma_start(
                    out=out_d[s0:s0 + 2].rearrange("l p c -> p l c"),
                    in_=uc)

            gts = {}
            # phase A: denominators + first conv tiles (no agg yet; the
            # pd PSUM pool coexists with pg_ps: 4 + 4 banks)
            with tc.tile_pool(name="pd_ps", bufs=4, space="PSUM") as pd_ps, \
                 tc.tile_pool(name="prf", bufs=3) as prf:
                ci = 0
                for x0 in range(0, W, 4):
                    xs = slice(x0, x0 + 4)
                    psd = pd_ps.tile([128, 4, 96], F32, tag="pd")
                    nc.tensor.matmul(psd, ones96b, Wc[:, xs, :],
                                     start=True, stop=False)
                    nc.tensor.matmul(psd, ones96b, WrT[:, xs, :],
                                     start=False, stop=True)
                    rf = prf.tile([128, 4, 96], F32, tag="rf")
                    nc.vector.reciprocal_approx_fast(out=rf, in_=psd)
                    nc.vector.tensor_copy(rr[:, xs, :], rf)
                    if x0 % 8 == 4:  # rr for x0..x0+7 ready
                        xs8 = slice(x0 - 4, x0 + 4)
                        nc.gpsimd.tensor_mul(Wc[:, xs8, :], Wc[:, xs8, :],
                                             rrv[:, xs8, :])
                        gts[ci] = emit_conv(ci)
                        ci += 1
                for y0 in range(0, H, 8):
                    ys = slice(y0, y0 + 8)
                    nc.gpsimd.tensor_mul(Wr[:, ys, :], Wr[:, ys, :],
                                         rrT[:, ys, :])
                    gts[ci] = emit_conv(ci)
                    ci += 1

            # phase B: steady pipeline; conv leads agg by the tiles staged
            # in phase A, so aggregation never waits on the gt copy
            with tc.tile_pool(name="pu_ps", bufs=2, space="PSUM") as pu_ps:
                for ai in range(len(tiles)):
                    if ci < len(tiles):
                        gts[ci] = emit_conv(ci)
                        ci += 1
                    emit_agg(ai, gts.pop(ai), pu_ps)

    nc.finalize()
    return nc


def _prep_shared(t_w, t_b, f_w, f_b, g_w, g_b, inc_w, inc_b):
    bf = ml_dtypes.bfloat16
    f8 = ml_dtypes.float8_e4m3
    m_w = inc_w @ g_w  # (C_IN, C_IN) fold: inc(g(.)) == M @ .
    s = TSCALE
    d = {
        "tf_wT": np.ascontiguousarray(
            np.concatenate([t_w.T * s, f_w.T * s], axis=1)
            .reshape(KC, 128, 128).transpose(1, 0, 2)).astype(f8),
        "tf_wT2": np.ascontiguousarray(
            np.concatenate([f_w.T * s, t_w.T * s], axis=1)
            .reshape(KC, 128, 128).transpose(1, 0, 2)).astype(f8),
        "m_wT": np.ascontiguousarray(
            (m_w.T * MSCALE).reshape(KC, 128, C_OUT)
            .transpose(1, 0, 2)).astype(f8),
        "tf_b": (np.concatenate([t_b, f_b]) * s).reshape(128, 1)
            .astype(np.float32),
        "tf_b2": (np.concatenate([f_b, t_b]) * s).reshape(128, 1)
            .astype(np.float32),
        "mask": (1.0 - np.eye(96)).astype(bf),
        "ones96b": np.ones((96, 128), bf),
    }
    comb_b = (inc_b + inc_w @ g_b).astype(np.float32)  # host-side bias
    return d, comb_b


def _make_in_maps(inputs):
    x = np.asarray(inputs["x"], dtype=np.float32)
    shared, comb_b = _prep_shared(*[
        np.asarray(inputs[k], np.float32)
        for k in ("t_w", "t_b", "f_w", "f_b", "g_w", "g_b", "inc_w", "inc_b")])
    _cache["comb_b"] = comb_b
    _cache["x"] = x
    f8 = ml_dtypes.float8_e4m3
    in_maps = []
    for n in range(N):
        xi = x[n].reshape(KC, 128, HW)  # (4, 128, 9216)
        m = dict(shared)
        m["x_f8"] = np.ascontiguousarray(xi.transpose(1, 0, 2)).astype(f8)
        in_maps.append(m)
    return in_maps


def _post(results):
    x = _cache["x"]
    comb_b = _cache["comb_b"]
    inv = 1.0 / MSCALE
    out = np.empty((N, C_IN, H, W), np.float32)
    for n in range(N):
        row = results[n]["out_row"].astype(np.float32)  # (H, W, C)
        col = results[n]["out_col"].astype(np.float32)  # (W, H, C)
        agg = row.transpose(2, 0, 1) + col.transpose(2, 1, 0)
        out[n] = x[n] + comb_b[:, None, None] + agg * inv
    return out


def kernel(x, t_w, t_b, f_w, f_b, g_w, g_b, inc_w, inc_b):
    in_maps = _make_in_maps(dict(
        x=x, t_w=t_w, t_b=t_b, f_w=f_w, f_b=f_b, g_w=g_w, g_b=g_b,
        inc_w=inc_w, inc_b=inc_b))

    if "nc" not in _cache:
        _cache["nc"] = build_program()
    res = run_bass_kernel_spmd(_cache["nc"], in_maps, core_ids=list(range(N)))
    return _post(res.results)


if __name__ == "__main__":
    rng = np.random.default_rng(0)
    ins = {
        "x": rng.standard_normal((N, C_IN, H, W), dtype=np.float32),
        "t_w": rng.standard_normal((C_INNER, C_IN), dtype=np.float32) * 0.02,
        "t_b": np.zeros(C_INNER, np.float32),
        "f_w": rng.standard_normal((C_INNER, C_IN), dtype=np.float32) * 0.02,
        "f_b": np.zeros(C_INNER, np.float32),
        "g_w": rng.standard_normal((C_OUT, C_IN), dtype=np.float32) * 0.02,
        "g_b": np.zeros(C_OUT, np.float32),
        "inc_w": rng.standard_normal((C_IN, C_OUT), dtype=np.float32) * 0.02,
        "inc_b": np.zeros(C_IN, np.float32),
    }
    y = kernel(**ins)
    print(y.shape, y.dtype)


# revision 14
# speedup vs baseline: 1.1186x; 1.0098x over previous
"""Criss-cross attention (CCNet) kernel for 8 TRN2 NeuronCores.

Data-parallel over batch N=8: one image per core. Per image (512ch, 96x96):
  t/f = 1x1 conv to 64ch -> row/col affinities -> exp -> denominators ->
  reciprocal folded into Wr/Wc -> aggregate g' = (inc_w@g_w)@x along rows
  and cols -> two spatial-major outputs, recombined with the residual on
  host.

Key folds vs the reference:
  - inc_w @ g_w is precomputed on host (M), so the g conv and inc conv
    become ONE conv of the aggregated input (softmax weights sum to 1, so
    g_b folds into a host-side bias).
  - All convs run in fp8e4 DoubleRow (2 contraction chunks per matmul).
    x ships only as fp8. M is prescaled by 512 (outputs divided on host);
    t/f weights+biases are prescaled by 256 and the exp's free input
    scale divides the affinities by 256^2.
  - softmax normalization (1/denominator) is multiplied into the attention
    weights before aggregation, so aggregation outputs are final.
  - Aggregation keeps the attention line stationary and streams g' (one
    N=512 matmul per line), producing spatial-major outputs.
  - residual + biases are added on host; outputs ship as bf16.
"""

import sys

sys.path.insert(0, "/opt/trn_rl_repo")

from contextlib import ExitStack

import numpy as np
import ml_dtypes

import concourse.bass as bass
import concourse.bacc as bacc
import concourse.tile as tile
from concourse import mybir
from concourse.bass_utils import run_bass_kernel_spmd

BF16 = mybir.dt.bfloat16
F32 = mybir.dt.float32
FP8 = mybir.dt.float8e4
AF = mybir.ActivationFunctionType
DR = mybir.MatmulPerfMode.DoubleRow

N, C_IN, C_INNER, C_OUT, H, W = 8, 512, 64, 512, 96, 96
HW = H * W  # 9216
KC = C_IN // 128  # 4 contraction chunks
MSCALE = 512.0   # fp8 prescale of M = inc_w @ g_w
TSCALE = 256.0   # fp8 prescale of t/f weights+biases

_cache = {}


def build_program():
    nc = bacc.Bacc()

    # ---- DRAM I/O ----
    xf8_d = nc.dram_tensor("x_f8", (128, KC, HW), FP8, kind="ExternalInput")
    tfw_d = nc.dram_tensor("tf_wT", (128, KC, 128), FP8, kind="ExternalInput")
    tfw2_d = nc.dram_tensor("tf_wT2", (128, KC, 128), FP8, kind="ExternalInput")
    mw_d = nc.dram_tensor("m_wT", (128, KC, C_OUT), FP8, kind="ExternalInput")
    tfb_d = nc.dram_tensor("tf_b", (128, 1), F32, kind="ExternalInput")
    tfb2_d = nc.dram_tensor("tf_b2", (128, 1), F32, kind="ExternalInput")
    mask_d = nc.dram_tensor("mask", (96, 96), BF16, kind="ExternalInput")
    ones96b_d = nc.dram_tensor("ones96b", (96, 128), BF16, kind="ExternalInput")
    # spatial-major outputs: [line, pos-in-line, channel]
    ocol_d = nc.dram_tensor("out_col", (W, H, C_OUT), BF16, kind="ExternalOutput")
    orow_d = nc.dram_tensor("out_row", (H, W, C_OUT), BF16, kind="ExternalOutput")

    with ExitStack() as ctx:
        tc = ctx.enter_context(tile.TileContext(nc))
        p0 = ctx.enter_context(tc.tile_pool(name="p0", bufs=1))

        # ---- persistent tiles ----
        Xf8 = p0.tile([128, KC, H, W], FP8)  # channel-major image, fp8
        ones96b = p0.tile([96, 128], BF16)
        mask = p0.tile([96, 96], BF16)
        mw = p0.tile([128, KC, C_OUT], FP8)
        TFb = p0.tile([128, 2, H, W], BF16)  # t/f activations (doubled, x256)
        Wr = p0.tile([96, H, W], BF16)       # exp row affin: Wr[i, y, x]
        Wc = p0.tile([96, W, H], BF16)       # exp col affin: Wc[j, x, y]
        rr = p0.tile([128, W, H], BF16)      # 1/denominator, [*, x, y] bcast

        nc.sync.dma_start(out=ones96b, in_=ones96b_d[:])
        nc.sync.dma_start(out=mask, in_=mask_d[:])
        nc.sync.dma_start(out=mw, in_=mw_d[:])
        xv8 = xf8_d[:].rearrange("p a (h w) -> p a h w", h=H)

        T = TFb[0:64, 0]
        F = TFb[0:64, 1]
        T2 = TFb[64:128, 0]
        F2 = TFb[64:128, 1]

        # ---- phase 1: t/f conv (fp8 DoubleRow, values x TSCALE) ----
        with tc.tile_pool(name="pe", bufs=1) as pe, \
             tc.tile_pool(name="ptf_ps", bufs=2, space="PSUM") as ptf_ps:
            tfw = pe.tile([128, KC, 128], FP8)
            tfw2 = pe.tile([128, KC, 128], FP8)
            tb = pe.tile([64, 1], F32)
            fb = pe.tile([64, 1], F32)
            tfbF = pe.tile([128, 1], F32)
            tb2 = pe.tile([128, 1], F32)
            nc.sync.dma_start(out=tfw, in_=tfw_d[:])
            nc.sync.dma_start(out=tfw2, in_=tfw2_d[:])
            nc.sync.dma_start(out=tb, in_=tfb_d[0:64])
            nc.sync.dma_start(out=fb, in_=tfb_d[64:128])
            nc.sync.dma_start(out=tfbF, in_=tfb_d[:])
            nc.sync.dma_start(out=tb2, in_=tfb2_d[:])
            for q in range(8):
                for k in range(KC):
                    nc.sync.dma_start(
                        out=Xf8[:, k, q * 12:(q + 1) * 12, :],
                        in_=xv8[:, k, q * 12:(q + 1) * 12, :])

            Xflat = Xf8.rearrange("p a h w -> p a (h w)")
            Tflat = TFb.rearrange("p c h w -> p c (h w)")
            for b in range(HW // 512):
                sl = slice(b * 512, (b + 1) * 512)
                pst = ptf_ps.tile([128, 512], F32, tag="pt")
                psf = ptf_ps.tile([128, 512], F32, tag="pf")
                for k in range(2):
                    nc.tensor.matmul(
                        pst, tfw[:, 2 * k:2 * k + 2, :],
                        Xflat[:, 2 * k:2 * k + 2, sl],
                        start=(k == 0), stop=(k == 1), perf_mode=DR)
                for k in range(2):
                    nc.tensor.matmul(
                        psf, tfw2[:, 2 * k:2 * k + 2, :],
                        Xflat[:, 2 * k:2 * k + 2, sl],
                        start=(k == 0), stop=(k == 1), perf_mode=DR)
                # pst = [t; f] stacked, psf = [f; t] stacked
                nc.scalar.activation(Tflat[0:64, 0, sl], pst[0:64],
                                     AF.Identity, bias=tb)
                nc.vector.tensor_scalar_add(Tflat[64:128, 1, sl], pst[64:128],
                                            tfbF[64:128])
                nc.scalar.activation(Tflat[0:64, 1, sl], psf[0:64],
                                     AF.Identity, bias=fb)
                nc.vector.tensor_scalar_add(Tflat[64:128, 0, sl], psf[64:128],
                                            tb2[64:128])

        # ---- phase 2: affinities + exp (+ mask on cols) ----
        # Wr[i, y, x] = exp(sum_c f[c,y,i] t[c,y,x] / TSCALE^2); 8-row blocks.
        ESC = 1.0 / (TSCALE * TSCALE)
        Wr8 = Wr.rearrange("i (b four two) w -> i b four two w", four=4, two=2)
        Wc8 = Wc.rearrange("j (b four two) y -> j b four two y", four=4, two=2)
        with tc.tile_pool(name="pe_ps", bufs=2, space="PSUM") as pe_ps:
            for y0 in range(0, H, 8):
                psA = pe_ps.tile([96, 4, 96], F32, tag="peA")
                psB = pe_ps.tile([96, 4, 96], F32, tag="peB")
                for h in range(4):
                    nc.tensor.matmul(psA[:, h, :], F[:, y0 + 2 * h, :],
                                     T[:, y0 + 2 * h, :], start=True, stop=True)
                    nc.tensor.matmul(psB[:, h, :], F2[:, y0 + 2 * h + 1, :],
                                     T2[:, y0 + 2 * h + 1, :],
                                     start=True, stop=True)
                b = y0 // 8
                nc.scalar.activation(Wr8[:, b, :, 0, :], psA, AF.Exp, scale=ESC)
                nc.scalar.activation(Wr8[:, b, :, 1, :], psB, AF.Exp, scale=ESC)
            # col affinities: Wc[j, x, y] = exp(.../TSCALE^2), j != y
            mb8 = bass.AP(tensor=mask.tensor, offset=mask.offset,
                          ap=[mask.ap[0], [0, 8], mask.ap[1]])
            for x0 in range(0, W, 8):
                psA = pe_ps.tile([96, 4, 96], F32, tag="peA")
                psB = pe_ps.tile([96, 4, 96], F32, tag="peB")
                for h in range(4):
                    nc.tensor.matmul(psA[:, h, :], F[:, :, x0 + 2 * h],
                                     T[:, :, x0 + 2 * h], start=True, stop=True)
                    nc.tensor.matmul(psB[:, h, :], F2[:, :, x0 + 2 * h + 1],
                                     T2[:, :, x0 + 2 * h + 1],
                                     start=True, stop=True)
                b = x0 // 8
                nc.scalar.activation(Wc8[:, b, :, 0, :], psA, AF.Exp, scale=ESC)
                nc.scalar.activation(Wc8[:, b, :, 1, :], psB, AF.Exp, scale=ESC)
                wcs = Wc[:, x0:x0 + 8, :]
                nc.gpsimd.tensor_mul(wcs, wcs, mb8)

        # ---- phases 3-5: denominators -> 1/D -> fold into W, overlapped
        # with the g' conv + aggregation pipeline (agg lags conv by LAG
        # tiles so the PSUM->SBUF copy never blocks the tensor queue) ----
        WrT = Wr.rearrange("i h w -> i w h")
        rrv = rr[0:96]
        rrT = rrv.rearrange("p x y -> p y x")

        tiles = [("col", s0) for s0 in range(0, 96, 2)] + \
                [("row", s0) for s0 in range(0, 96, 2)]

        with tc.tile_pool(name="pg", bufs=26) as pg, \
             tc.tile_pool(name="pu", bufs=3) as pu, \
             tc.tile_pool(name="pg_ps", bufs=4, space="PSUM") as pg_ps:

            def emit_conv(idx):
                mode, s0 = tiles[idx]
                gt = pg.tile([96, 2, C_OUT], BF16, tag="gt")
                for i in range(2):
                    psg = pg_ps.tile([96, C_OUT], F32, tag="pg")
                    for k in range(2):
                        lhs = (Xf8[:, 2 * k:2 * k + 2, :, s0 + i]
                               if mode == "col"
                               else Xf8[:, 2 * k:2 * k + 2, s0 + i, :])
                        nc.tensor.matmul(psg, lhs,
                                         mw[:, 2 * k:2 * k + 2, :],
                                         start=(k == 0), stop=(k == 1),
                                         perf_mode=DR)
                    if i % 2 == 0:
                        nc.scalar.copy(gt[:, i, :], psg)
                    else:
                        nc.vector.tensor_copy(gt[:, i, :], psg)
                return gt

            def emit_agg(idx, gt, pu_ps):
                mode, s0 = tiles[idx]
                Wa = Wc if mode == "col" else Wr
                out_d = ocol_d if mode == "col" else orow_d
                psu2 = pu_ps.tile([96, 2, C_OUT], F32, tag="pu")
                for i in range(2):
                    nc.tensor.matmul(psu2[:, i, :], Wa[:, s0 + i, :],
                                     gt[:, i, :], start=True, stop=True)
                uc = pu.tile([96, 2, C_OUT], BF16, tag="uc")
                if idx % 2 == 1:
                    nc.scalar.copy(uc, psu2)
                else:
                    nc.vector.tensor_copy(uc, psu2)
                nc.sync.dma_start(
                    out=out_d[s0:s0 + 2].rearrange("l p c -> p l c"),
                    in_=uc)

            gts = {}
            # phase A: denominators + first conv tiles (no agg yet; the
            # pd PSUM pool coexists with pg_ps: 4 + 4 banks)
            with tc.tile_pool(name="pd_ps", bufs=4, space="PSUM") as pd_ps, \
                 tc.tile_pool(name="prf", bufs=3) as prf:
                ci = 0
                for x0 in range(0, W, 4):
                    xs = slice(x0, x0 + 4)
                    psd = pd_ps.tile([128, 4, 96], F32, tag="pd")
                    nc.tensor.matmul(psd, ones96b, Wc[:, xs, :],
                                     start=True, stop=False)
                    nc.tensor.matmul(psd, ones96b, WrT[:, xs, :],
                                     start=False, stop=True)
                    rf = prf.tile([128, 4, 96], F32, tag="rf")
                    nc.vector.reciprocal_approx_fast(out=rf, in_=psd)
                    nc.vector.tensor_copy(rr[:, xs, :], rf)
                    if x0 % 8 == 4:  # rr for x0..x0+7 ready
                        xs8 = slice(x0 - 4, x0 + 4)
                        nc.gpsimd.tensor_mul(Wc[:, xs8, :], Wc[:, xs8, :],
                                             rrv[:, xs8, :])
                        gts[ci] = emit_conv(ci)
                        ci += 1
                for y0 in range(0, H, 8):
                    ys = slice(y0, y0 + 8)
                    nc.gpsimd.tensor_mul(Wr[:, ys, :], Wr[:, ys, :],
                                         rrT[:, ys, :])
                    gts[ci] = emit_conv(ci)
                    ci += 1

            # phase B: steady pipeline; conv leads agg by the tiles staged
            # in phase A, so aggregation never waits on the gt copy
            with tc.tile_pool(name="pu_ps", bufs=2, space="PSUM") as pu_ps:
                for ai in range(len(tiles)):
                    if ci < len(tiles):
                        gts[ci] = emit_conv(ci)
                        ci += 1
                    emit_agg(ai, gts.pop(ai), pu_ps)

    nc.finalize()
    return nc


def _prep_shared(t_w, t_b, f_w, f_b, g_w, g_b, inc_w, inc_b):
    bf = ml_dtypes.bfloat16
    f8 = ml_dtypes.float8_e4m3
    m_w = inc_w @ g_w  # (C_IN, C_IN) fold: inc(g(.)) == M @ .
    s = TSCALE
    d = {
        "tf_wT": np.ascontiguousarray(
            np.concatenate([t_w.T * s, f_w.T * s], axis=1)
            .reshape(KC, 128, 128).transpose(1, 0, 2)).astype(f8),
        "tf_wT2": np.ascontiguousarray(
            np.concatenate([f_w.T * s, t_w.T * s], axis=1)
            .reshape(KC, 128, 128).transpose(1, 0, 2)).astype(f8),
        "m_wT": np.ascontiguousarray(
            (m_w.T * MSCALE).reshape(KC, 128, C_OUT)
            .transpose(1, 0, 2)).astype(f8),
        "tf_b": (np.concatenate([t_b, f_b]) * s).reshape(128, 1)
            .astype(np.float32),
        "tf_b2": (np.concatenate([f_b, t_b]) * s).reshape(128, 1)
            .astype(np.float32),
        "mask": (1.0 - np.eye(96)).astype(bf),
        "ones96b": np.ones((96, 128), bf),
    }
    comb_b = (inc_b + inc_w @ g_b).astype(np.float32)  # host-side bias
    return d, comb_b


def _make_in_maps(inputs):
    x = np.asarray(inputs["x"], dtype=np.float32)
    shared, comb_b = _prep_shared(*[
        np.asarray(inputs[k], np.float32)
        for k in ("t_w", "t_b", "f_w", "f_b", "g_w", "g_b", "inc_w", "inc_b")])
    _cache["comb_b"] = comb_b
    _cache["x"] = x
    f8 = ml_dtypes.float8_e4m3
    in_maps = []
    for n in range(N):
        xi = x[n].reshape(KC, 128, HW)  # (4, 128, 9216)
        m = dict(shared)
        m["x_f8"] = np.ascontiguousarray(xi.transpose(1, 0, 2)).astype(f8)
        in_maps.append(m)
    return in_maps


def _post(results):
    x = _cache["x"]
    comb_b = _cache["comb_b"]
    inv = 1.0 / MSCALE
    out = np.empty((N, C_IN, H, W), np.float32)
    for n in range(N):
        row = results[n]["out_row"].astype(np.float32)  # (H, W, C)
        col = results[n]["out_col"].astype(np.float32)  # (W, H, C)
        agg = row.transpose(2, 0, 1) + col.transpose(2, 1, 0)
        out[n] = x[n] + comb_b[:, None, None] + agg * inv
    return out


def kernel(x, t_w, t_b, f_w, f_b, g_w, g_b, inc_w, inc_b):
    in_maps = _make_in_maps(dict(
        x=x, t_w=t_w, t_b=t_b, f_w=f_w, f_b=f_b, g_w=g_w, g_b=g_b,
        inc_w=inc_w, inc_b=inc_b))

    if "nc" not in _cache:
        _cache["nc"] = build_program()
    res = run_bass_kernel_spmd(_cache["nc"], in_maps, core_ids=list(range(N)))
    return _post(res.results)


if __name__ == "__main__":
    rng = np.random.default_rng(0)
    ins = {
        "x": rng.standard_normal((N, C_IN, H, W), dtype=np.float32),
        "t_w": rng.standard_normal((C_INNER, C_IN), dtype=np.float32) * 0.02,
        "t_b": np.zeros(C_INNER, np.float32),
        "f_w": rng.standard_normal((C_INNER, C_IN), dtype=np.float32) * 0.02,
        "f_b": np.zeros(C_INNER, np.float32),
        "g_w": rng.standard_normal((C_OUT, C_IN), dtype=np.float32) * 0.02,
        "g_b": np.zeros(C_OUT, np.float32),
        "inc_w": rng.standard_normal((C_IN, C_OUT), dtype=np.float32) * 0.02,
        "inc_b": np.zeros(C_IN, np.float32),
    }
    y = kernel(**ins)
    print(y.shape, y.dtype)


# revision 21
# speedup vs baseline: 1.1339x; 1.0136x over previous
"""Criss-cross attention (CCNet) kernel for 8 TRN2 NeuronCores.

Data-parallel over batch N=8: one image per core. Per image (512ch, 96x96):
  t/f = 1x1 conv to 64ch -> row/col affinities -> exp -> denominators ->
  reciprocal folded into Wr/Wc -> aggregate g' = (inc_w@g_w)@x along rows
  and cols -> two spatial-major outputs, recombined with the residual on
  host.

Key folds vs the reference:
  - inc_w @ g_w is precomputed on host (M), so the g conv and inc conv
    become ONE conv of the aggregated input (softmax weights sum to 1, so
    g_b folds into a host-side bias).
  - All convs run in fp8e4 DoubleRow (2 contraction chunks per matmul).
    x ships only as fp8. M is prescaled by 512 (outputs divided on host);
    t/f weights+biases are prescaled by 256 and the exp's free input
    scale divides the affinities by 256^2.
  - softmax normalization (1/denominator) is multiplied into the attention
    weights before aggregation, so aggregation outputs are final.
  - Aggregation keeps the attention line stationary and streams g' (one
    N=512 matmul per line), producing spatial-major outputs.
  - residual + biases are added on host; outputs ship as bf16.
"""

import sys

sys.path.insert(0, "/opt/trn_rl_repo")

from contextlib import ExitStack

import numpy as np
import ml_dtypes

import concourse.bass as bass
import concourse.bacc as bacc
import concourse.tile as tile
from concourse import mybir
from concourse.bass_utils import run_bass_kernel_spmd

BF16 = mybir.dt.bfloat16
F32 = mybir.dt.float32
FP8 = mybir.dt.float8e4
AF = mybir.ActivationFunctionType
DR = mybir.MatmulPerfMode.DoubleRow

N, C_IN, C_INNER, C_OUT, H, W = 8, 512, 64, 512, 96, 96
HW = H * W  # 9216
KC = C_IN // 128  # 4 contraction chunks
MSCALE = 512.0   # fp8 prescale of M = inc_w @ g_w
TSCALE = 256.0   # fp8 prescale of t/f weights+biases

_cache = {}


def build_program():
    nc = bacc.Bacc()

    # ---- DRAM I/O ----
    xf8_d = nc.dram_tensor("x_f8", (128, KC, HW), FP8, kind="ExternalInput")
    tfw_d = nc.dram_tensor("tf_wT", (128, KC, 128), FP8, kind="ExternalInput")
    tfw2_d = nc.dram_tensor("tf_wT2", (128, KC, 128), FP8, kind="ExternalInput")
    mw_d = nc.dram_tensor("m_wT", (128, KC, C_OUT), FP8, kind="ExternalInput")
    tfb_d = nc.dram_tensor("tf_b", (128, 1), F32, kind="ExternalInput")
    tfb2_d = nc.dram_tensor("tf_b2", (128, 1), F32, kind="ExternalInput")
    mask_d = nc.dram_tensor("mask", (96, 96), BF16, kind="ExternalInput")
    ones96b_d = nc.dram_tensor("ones96b", (96, 128), BF16, kind="ExternalInput")
    # spatial-major outputs: [line, pos-in-line, channel]
    ocol_d = nc.dram_tensor("out_col", (W, H, C_OUT), BF16, kind="ExternalOutput")
    orow_d = nc.dram_tensor("out_row", (H, W, C_OUT), BF16, kind="ExternalOutput")

    with ExitStack() as ctx:
        tc = ctx.enter_context(tile.TileContext(nc))
        p0 = ctx.enter_context(tc.tile_pool(name="p0", bufs=1))

        # ---- persistent tiles ----
        Xf8 = p0.tile([128, KC, H, W], FP8)  # channel-major image, fp8
        ones96b = p0.tile([96, 128], BF16)
        mask = p0.tile([96, 96], BF16)
        mw = p0.tile([128, KC, C_OUT], FP8)
        TFb = p0.tile([128, 2, H, W], BF16)  # t/f activations (doubled, x256)
        Wr = p0.tile([96, H, W], BF16)       # exp row affin: Wr[i, y, x]
        Wc = p0.tile([96, W, H], BF16)       # exp col affin: Wc[j, x, y]
        WrX = p0.tile([96, W, H], BF16)      # Wr transposed: WrX[i, x, y]
        rr = p0.tile([128, W, H], BF16)      # 1/denominator, [*, x, y] bcast

        nc.sync.dma_start(out=ones96b, in_=ones96b_d[:])
        nc.sync.dma_start(out=mask, in_=mask_d[:])
        nc.sync.dma_start(out=mw, in_=mw_d[:])
        xv8 = xf8_d[:].rearrange("p a (h w) -> p a h w", h=H)

        T = TFb[0:64, 0]
        F = TFb[0:64, 1]
        T2 = TFb[64:128, 0]
        F2 = TFb[64:128, 1]

        # ---- phase 1: t/f conv (fp8 DoubleRow, values x TSCALE) ----
        with tc.tile_pool(name="pe", bufs=1) as pe, \
             tc.tile_pool(name="ptf_ps", bufs=2, space="PSUM") as ptf_ps:
            tfw = pe.tile([128, KC, 128], FP8)
            tfw2 = pe.tile([128, KC, 128], FP8)
            tb = pe.tile([64, 1], F32)
            fb = pe.tile([64, 1], F32)
            tfbF = pe.tile([128, 1], F32)
            tb2 = pe.tile([128, 1], F32)
            nc.sync.dma_start(out=tfw, in_=tfw_d[:])
            nc.sync.dma_start(out=tfw2, in_=tfw2_d[:])
            nc.sync.dma_start(out=tb, in_=tfb_d[0:64])
            nc.sync.dma_start(out=fb, in_=tfb_d[64:128])
            nc.sync.dma_start(out=tfbF, in_=tfb_d[:])
            nc.sync.dma_start(out=tb2, in_=tfb2_d[:])
            for q in range(8):
                for k in range(KC):
                    nc.sync.dma_start(
                        out=Xf8[:, k, q * 12:(q + 1) * 12, :],
                        in_=xv8[:, k, q * 12:(q + 1) * 12, :])

            Xflat = Xf8.rearrange("p a h w -> p a (h w)")
            Tflat = TFb.rearrange("p c h w -> p c (h w)")
            for b2 in range(HW // 1024):
                sl = slice(b2 * 1024, (b2 + 1) * 1024)
                pst = ptf_ps.tile([128, 2, 512], F32, tag="pt")
                psf = ptf_ps.tile([128, 2, 512], F32, tag="pf")
                for j in range(2):
                    slj = slice(b2 * 1024 + j * 512, b2 * 1024 + (j + 1) * 512)
                    for k in range(2):
                        nc.tensor.matmul(
                            pst[:, j, :], tfw[:, 2 * k:2 * k + 2, :],
                            Xflat[:, 2 * k:2 * k + 2, slj],
                            start=(k == 0), stop=(k == 1), perf_mode=DR)
                    for k in range(2):
                        nc.tensor.matmul(
                            psf[:, j, :], tfw2[:, 2 * k:2 * k + 2, :],
                            Xflat[:, 2 * k:2 * k + 2, slj],
                            start=(k == 0), stop=(k == 1), perf_mode=DR)
                # pst = [t; f] stacked, psf = [f; t] stacked
                pstf = pst.rearrange("p j c -> p (j c)")
                psff = psf.rearrange("p j c -> p (j c)")
                nc.scalar.activation(Tflat[0:64, 0, sl], pstf[0:64],
                                     AF.Identity, bias=tb)
                nc.vector.tensor_scalar_add(Tflat[64:128, 1, sl], pstf[64:128],
                                            tfbF[64:128])
                nc.scalar.activation(Tflat[0:64, 1, sl], psff[0:64],
                                     AF.Identity, bias=fb)
                nc.vector.tensor_scalar_add(Tflat[64:128, 0, sl], psff[64:128],
                                            tb2[64:128])

        # ---- phase 2: affinities + exp (+ mask on cols) ----
        # Wr[i, y, x] = exp(sum_c f[c,y,i] t[c,y,x] / TSCALE^2); 8-row blocks.
        ESC = 1.0 / (TSCALE * TSCALE)
        Wr8 = Wr.rearrange("i (b four two) w -> i b four two w", four=4, two=2)
        Wc8 = Wc.rearrange("j (b four two) y -> j b four two y", four=4, two=2)
        with tc.tile_pool(name="pe_ps", bufs=2, space="PSUM") as pe_ps:
            for y0 in range(0, H, 8):
                psA = pe_ps.tile([96, 4, 96], F32, tag="peA")
                psB = pe_ps.tile([96, 4, 96], F32, tag="peB")
                for h in range(4):
                    nc.tensor.matmul(psA[:, h, :], F[:, y0 + 2 * h, :],
                                     T[:, y0 + 2 * h, :], start=True, stop=True)
                    nc.tensor.matmul(psB[:, h, :], F2[:, y0 + 2 * h + 1, :],
                                     T2[:, y0 + 2 * h + 1, :],
                                     start=True, stop=True)
                b = y0 // 8
                nc.scalar.activation(Wr8[:, b, :, 0, :], psA, AF.Exp, scale=ESC)
                nc.scalar.activation(Wr8[:, b, :, 1, :], psB, AF.Exp, scale=ESC)
                # transposed copy for contiguous denominator reads (gpsimd
                # is idle here; strided read, contiguous-ish write)
                ys = slice(y0, y0 + 8)
                nc.gpsimd.tensor_copy(
                    WrX[:, :, ys],
                    Wr[:, ys, :].rearrange("i y x -> i x y"))
            # col affinities: Wc[j, x, y] = exp(.../TSCALE^2), j != y
            mb8 = bass.AP(tensor=mask.tensor, offset=mask.offset,
                          ap=[mask.ap[0], [0, 8], mask.ap[1]])
            for x0 in range(0, W, 8):
                psA = pe_ps.tile([96, 4, 96], F32, tag="peA")
                psB = pe_ps.tile([96, 4, 96], F32, tag="peB")
                for h in range(4):
                    nc.tensor.matmul(psA[:, h, :], F[:, :, x0 + 2 * h],
                                     T[:, :, x0 + 2 * h], start=True, stop=True)
                    nc.tensor.matmul(psB[:, h, :], F2[:, :, x0 + 2 * h + 1],
                                     T2[:, :, x0 + 2 * h + 1],
                                     start=True, stop=True)
                b = x0 // 8
                nc.scalar.activation(Wc8[:, b, :, 0, :], psA, AF.Exp, scale=ESC)
                nc.scalar.activation(Wc8[:, b, :, 1, :], psB, AF.Exp, scale=ESC)
                wcs = Wc[:, x0:x0 + 8, :]
                nc.gpsimd.tensor_mul(wcs, wcs, mb8)

        # ---- phases 3-5: denominators -> 1/D -> fold into W, overlapped
        # with the g' conv + aggregation pipeline (agg lags conv by the
        # tiles staged in phase A so the PSUM->SBUF copy never blocks) ----
        rrv = rr[0:96]
        rrT = rrv.rearrange("p x y -> p y x")

        tiles = [("col", s0) for s0 in range(0, 96, 2)] + \
                [("row", s0) for s0 in range(0, 96, 2)]

        with tc.tile_pool(name="pg", bufs=22) as pg, \
             tc.tile_pool(name="pu", bufs=3) as pu, \
             tc.tile_pool(name="pg_ps", bufs=4, space="PSUM") as pg_ps:

            def emit_conv(idx):
                mode, s0 = tiles[idx]
                gt = pg.tile([96, 2, C_OUT], BF16, tag="gt")
                for i in range(2):
                    psg = pg_ps.tile([96, C_OUT], F32, tag="pg")
                    for k in range(2):
                        lhs = (Xf8[:, 2 * k:2 * k + 2, :, s0 + i]
                               if mode == "col"
                               else Xf8[:, 2 * k:2 * k + 2, s0 + i, :])
                        nc.tensor.matmul(psg, lhs,
                                         mw[:, 2 * k:2 * k + 2, :],
                                         start=(k == 0), stop=(k == 1),
                                         perf_mode=DR)
                    if i % 2 == 0:
                        nc.scalar.copy(gt[:, i, :], psg)
                    else:
                        nc.vector.tensor_copy(gt[:, i, :], psg)
                return gt

            def emit_agg(idx, gt, pu_ps):
                mode, s0 = tiles[idx]
                Wa = Wc if mode == "col" else Wr
                out_d = ocol_d if mode == "col" else orow_d
                psu2 = pu_ps.tile([96, 2, C_OUT], F32, tag="pu")
                for i in range(2):
                    nc.tensor.matmul(psu2[:, i, :], Wa[:, s0 + i, :],
                                     gt[:, i, :], start=True, stop=True)
                uc = pu.tile([96, 2, C_OUT], BF16, tag="uc")
                if idx % 2 == 1:
                    nc.scalar.copy(uc, psu2)
                else:
                    nc.vector.tensor_copy(uc, psu2)
                nc.sync.dma_start(
                    out=out_d[s0:s0 + 2].rearrange("l p c -> p l c"),
                    in_=uc)

            gts = {}
            # phase A: denominators + first conv tiles (no agg yet; the
            # pd PSUM pool coexists with pg_ps: 4 + 4 banks)
            with tc.tile_pool(name="pd_ps", bufs=4, space="PSUM") as pd_ps, \
                 tc.tile_pool(name="prf", bufs=3) as prf:
                ci = 0
                for x0 in range(0, W, 4):
                    xs = slice(x0, x0 + 4)
                    psd = pd_ps.tile([128, 4, 96], F32, tag="pd")
                    nc.tensor.matmul(psd, ones96b, Wc[:, xs, :],
                                     start=True, stop=False)
                    nc.tensor.matmul(psd, ones96b, WrX[:, xs, :],
                                     start=False, stop=True)
                    rf = prf.tile([128, 4, 96], F32, tag="rf")
                    nc.vector.reciprocal_approx_fast(out=rf, in_=psd)
                    nc.scalar.copy(rr[:, xs, :], rf)
                    if x0 % 8 == 4:  # rr for x0..x0+7 ready
                        xs8 = slice(x0 - 4, x0 + 4)
                        nc.gpsimd.tensor_mul(Wc[:, xs8, :], Wc[:, xs8, :],
                                             rrv[:, xs8, :])
                        gts[ci] = emit_conv(ci)
                        ci += 1
                for y0 in range(0, H, 8):
                    ys = slice(y0, y0 + 8)
                    nc.gpsimd.tensor_mul(Wr[:, ys, :], Wr[:, ys, :],
                                         rrT[:, ys, :])
                    if y0 % 12 == 0:
                        gts[ci] = emit_conv(ci)
                        ci += 1

            # phase B: steady pipeline; conv leads agg by the tiles staged
            # in phase A, so aggregation never waits on the gt copy
            with tc.tile_pool(name="pu_ps", bufs=2, space="PSUM") as pu_ps:
                for ai in range(len(tiles)):
                    if ci < len(tiles):
                        gts[ci] = emit_conv(ci)
                        ci += 1
                    emit_agg(ai, gts.pop(ai), pu_ps)

    nc.finalize()
    return nc


def _prep_shared(t_w, t_b, f_w, f_b, g_w, g_b, inc_w, inc_b):
    bf = ml_dtypes.bfloat16
    f8 = ml_dtypes.float8_e4m3
    m_w = inc_w @ g_w  # (C_IN, C_IN) fold: inc(g(.)) == M @ .
    s = TSCALE
    d = {
        "tf_wT": np.ascontiguousarray(
            np.concatenate([t_w.T * s, f_w.T * s], axis=1)
            .reshape(KC, 128, 128).transpose(1, 0, 2)).astype(f8),
        "tf_wT2": np.ascontiguousarray(
            np.concatenate([f_w.T * s, t_w.T * s], axis=1)
            .reshape(KC, 128, 128).transpose(1, 0, 2)).astype(f8),
        "m_wT": np.ascontiguousarray(
            (m_w.T * MSCALE).reshape(KC, 128, C_OUT)
            .transpose(1, 0, 2)).astype(f8),
        "tf_b": (np.concatenate([t_b, f_b]) * s).reshape(128, 1)
            .astype(np.float32),
        "tf_b2": (np.concatenate([f_b, t_b]) * s).reshape(128, 1)
            .astype(np.float32),
        "mask": (1.0 - np.eye(96)).astype(bf),
        "ones96b": np.ones((96, 128), bf),
    }
    comb_b = (inc_b + inc_w @ g_b).astype(np.float32)  # host-side bias
    return d, comb_b


def _make_in_maps(inputs):
    x = np.asarray(inputs["x"], dtype=np.float32)
    shared, comb_b = _prep_shared(*[
        np.asarray(inputs[k], np.float32)
        for k in ("t_w", "t_b", "f_w", "f_b", "g_w", "g_b", "inc_w", "inc_b")])
    _cache["comb_b"] = comb_b
    _cache["x"] = x
    f8 = ml_dtypes.float8_e4m3
    in_maps = []
    for n in range(N):
        xi = x[n].reshape(KC, 128, HW)  # (4, 128, 9216)
        m = dict(shared)
        m["x_f8"] = np.ascontiguousarray(xi.transpose(1, 0, 2)).astype(f8)
        in_maps.append(m)
    return in_maps


def _post(results):
    x = _cache["x"]
    comb_b = _cache["comb_b"]
    inv = 1.0 / MSCALE
    out = np.empty((N, C_IN, H, W), np.float32)
    for n in range(N):
        row = results[n]["out_row"].astype(np.float32)  # (H, W, C)
        col = results[n]["out_col"].astype(np.float32)  # (W, H, C)
        agg = row.transpose(2, 0, 1) + col.transpose(2, 1, 0)
        out[n] = x[n] + comb_b[:, None, None] + agg * inv
    return out


def kernel(x, t_w, t_b, f_w, f_b, g_w, g_b, inc_w, inc_b):
    in_maps = _make_in_maps(dict(
        x=x, t_w=t_w, t_b=t_b, f_w=f_w, f_b=f_b, g_w=g_w, g_b=g_b,
        inc_w=inc_w, inc_b=inc_b))

    if "nc" not in _cache:
        _cache["nc"] = build_program()
    res = run_bass_kernel_spmd(_cache["nc"], in_maps, core_ids=list(range(N)))
    return _post(res.results)


if __name__ == "__main__":
    rng = np.random.default_rng(0)
    ins = {
        "x": rng.standard_normal((N, C_IN, H, W), dtype=np.float32),
        "t_w": rng.standard_normal((C_INNER, C_IN), dtype=np.float32) * 0.02,
        "t_b": np.zeros(C_INNER, np.float32),
        "f_w": rng.standard_normal((C_INNER, C_IN), dtype=np.float32) * 0.02,
        "f_b": np.zeros(C_INNER, np.float32),
        "g_w": rng.standard_normal((C_OUT, C_IN), dtype=np.float32) * 0.02,
        "g_b": np.zeros(C_OUT, np.float32),
        "inc_w": rng.standard_normal((C_IN, C_OUT), dtype=np.float32) * 0.02,
        "inc_b": np.zeros(C_IN, np.float32),
    }
    y = kernel(**ins)
    print(y.shape, y.dtype)
